# revision 16
# baseline (speedup 1.0000x reference)
"""LLaMA attention block on 8 Trainium2 NeuronCores (Bass/Tile).

Problem: x[32,256,2048], wq/wk/wv/wo[2048,2048] fp32.
  q/k/v = x@W.T (per-head RoPE on q,k), causal softmax attention, y@wo.T.

Strategy (v3):
- Data-parallel over batch: 8 cores x 4 batch elements (1024 tokens/core).
- bf16 operands everywhere on the matmul path (weights, x, q/k/v, exp-scores,
  y), fp32 PSUM accumulation. Measured end-to-end error vs fp64 ~6e-3,
  well under the 2e-2 gate. PE streams 1 col/cycle for bf16 == f32r, so this
  costs no PE time but halves DMA and SBUF traffic.
- No DRAM spill: q/k (rotated) live in small rotating SBUF pools; v and y
  (bf16, 4MB each) persist in SBUF. Per-head loop interleaves
  {q-proj, k-proj, attention} so attention overlaps the next head's
  projection matmuls and the PE never drains.
- All pools are allocated up-front at disjoint SBUF addresses: scoped/reused
  address ranges were observed (TimelineSim) to serialize weight prefetch
  behind the previous phase (a 4-8us PE gap per phase boundary).
- x is loaded as 16 separate tiles so the first V matmuls start after ~2us.
- Weights are pre-tiled on the host into the exact SBUF layout so every
  weight DMA is fully contiguous.
- "Transposed" activation layout on-chip ([contraction, tokens]); softmax
  reduction via ones-matmul; normalization via K=1 broadcast matmul.
- RoPE: rotate-half as one 128x128 matmul per tile; elementwise
  (q*cos + rq*sin) in bf16 on DVE (4x mode); PSUM drains on ScalarE.
- _build(loop_n=N) wraps the whole body in a tc.For_i hardware loop: one
  NEFF execution runs the kernel N times back-to-back. Used by test.py to
  measure pure on-device time as a slope between two loop counts,
  cancelling host/axon dispatch latency.
"""
import sys
sys.path.insert(0, '/opt/trn_rl_repo')
import math
import numpy as np
import ml_dtypes

import concourse.bass as bass
import concourse.bass_isa as bass_isa
import concourse.bacc as bacc
import concourse.mybir as mybir
import concourse.tile as tile
from concourse.bass_utils import run_bass_kernel_spmd

B, T, C = 32, 256, 2048
H, D = 16, 128
NCORES = 8
BPC = B // NCORES           # 4 batches per core
PAIRS = BPC // 2            # 2 batch-pairs (N=512 matmuls)
KT = C // 128               # 16 contraction tiles
SCALE = 1.0 / math.sqrt(D)

F32 = mybir.dt.float32
F32R = mybir.dt.float32r
BF16 = mybir.dt.bfloat16
AF = mybir.ActivationFunctionType
NPBF = ml_dtypes.bfloat16

_CACHE = {}


def _build(loop_n: int = 1):
    nc = bacc.Bacc("TRN2", target_bir_lowering=False, debug=False, num_devices=1)
    dt_in = {
        "xt": ([KT, 128, BPC * T], BF16),      # [kt, p, (b t)]
        "wq": ([16, 128, KT * 128], BF16),     # [o_blk, p, (kt n)]
        "wk": ([16, 128, KT * 128], BF16),
        "wv": ([4, 128, KT * 512], BF16),      # [og, p, (kt n)]
        "wo": ([16, 128, KT * 128], BF16),
        "rmatT": ([128, 128], BF16),
        "cos2": ([128, 512], BF16),
        "sin2": ([128, 512], BF16),
        "mask2": ([128, 512], BF16),
        "onescol": ([128, 1], BF16),
        "ones1x": ([1, 128], F32R),
    }
    aps = {n: nc.dram_tensor(n, s, d, kind="ExternalInput").ap()
           for n, (s, d) in dt_in.items()}
    out_d = nc.dram_tensor("out", [C, BPC * T], F32, kind="ExternalOutput").ap()

    with tile.TileContext(nc) as tc:
        from contextlib import ExitStack
        with ExitStack() as top:
            # ---- constants (loaded once, outside any timing loop) ----
            cpool = top.enter_context(tc.tile_pool(name="const", bufs=1))
            ct = {}
            for n in ("rmatT", "cos2", "sin2", "mask2", "onescol", "ones1x"):
                shape = dt_in[n][0]
                ct[n] = cpool.tile(list(shape), dt_in[n][1], tag=n, name=n)
                nc.sync.dma_start(ct[n][:], aps[n][:])

            # ---- all pools up-front: disjoint SBUF regions ----
            xpool = top.enter_context(tc.tile_pool(name="xt", bufs=1))
            xtiles = [xpool.tile([128, BPC * T], BF16, tag=f"xt{kt}",
                                 name=f"xt{kt}") for kt in range(KT)]
            vpool = top.enter_context(tc.tile_pool(name="v", bufs=1))
            vsb = [vpool.tile([128, C], BF16, tag=f"v{i}", name=f"v{i}")
                   for i in range(BPC * 2)]       # [tok(128), all head dims]
            ypool = top.enter_context(tc.tile_pool(name="y", bufs=1))
            ysb = [ypool.tile([128, BPC * T], BF16, tag=f"y{h}", name=f"y{h}")
                   for h in range(H)]             # [head dims(128), tok]
            qkpool = top.enter_context(tc.tile_pool(name="qk", bufs=4))
            st = top.enter_context(tc.tile_pool(name="st", bufs=2))
            st3 = top.enter_context(tc.tile_pool(name="st3", bufs=3))
            vw = top.enter_context(tc.tile_pool(name="vw", bufs=4))
            qkw = top.enter_context(tc.tile_pool(name="qkw", bufs=3))
            ow = top.enter_context(tc.tile_pool(name="ow", bufs=2))

            # ---- PSUM pools: exactly 8 banks ----
            pp = top.enter_context(tc.tile_pool(name="pp", bufs=3, space="PSUM"))
            rp = top.enter_context(tc.tile_pool(name="rp", bufs=1, space="PSUM"))
            scp = top.enter_context(tc.tile_pool(name="scp", bufs=2, space="PSUM"))
            yp = top.enter_context(tc.tile_pool(name="yp", bufs=2, space="PSUM"))

            def emit_body():
                # First V matmul needs wv[0] first-half + xt[0]; queue those
                # bytes ahead of the rest of x so the PE starts ~3.5us in.
                wv_first = vw.tile([128, 8 * 512], BF16, tag="wvb")
                nc.sync.dma_start(wv_first[:], aps["wv"][0, :, 0:4096])
                for kt in range(KT):
                    nc.sync.dma_start(xtiles[kt][:], aps["xt"][kt])

                # ---- V projection: v in [tok, C] layout, 4-head groups,
                #      each og's weight split in two tiles for finer DMA deps
                for og in range(4):
                    wbs = []
                    for half in range(2):
                        if og == 0 and half == 0:
                            wbs.append(wv_first)
                            continue
                        wb = vw.tile([128, 8 * 512], BF16, tag="wvb")
                        nc.sync.dma_start(
                            wb[:], aps["wv"][og, :, half * 4096:(half + 1) * 4096])
                        wbs.append(wb)
                    for tt in range(BPC * 2):
                        ps = pp.tile([128, 512], F32, tag="pp")
                        for kt in range(KT):
                            nc.tensor.matmul(
                                ps[:],
                                xtiles[kt][:, tt * 128:(tt + 1) * 128],
                                wbs[kt // 8][:, (kt % 8) * 512:(kt % 8 + 1) * 512],
                                start=(kt == 0), stop=(kt == KT - 1))
                        nc.scalar.activation(
                            vsb[tt][:, og * 512:(og + 1) * 512], ps[:], AF.Identity)

                # ---- per-head: q-proj+rope, k-proj+rope, attention ----
                def proj_rope(wname, h, tag):
                    wb = qkw.tile([128, KT * 128], BF16, tag="wqk")
                    nc.sync.dma_start(wb[:], aps[wname][h])
                    dst = qkpool.tile([128, BPC * T], BF16, tag=tag)
                    # kt-outer so each 128-col weight slice is loaded into the
                    # PE once and streamed against both batch-pairs.
                    pss = [pp.tile([128, 512], F32, tag="pp", name=f"ps{pr}")
                           for pr in range(PAIRS)]
                    for kt in range(KT):
                        for pr in range(PAIRS):
                            nc.tensor.matmul(
                                pss[pr][:], wb[:, kt * 128:(kt + 1) * 128],
                                xtiles[kt][:, pr * 512:(pr + 1) * 512],
                                start=(kt == 0), stop=(kt == KT - 1))
                    for pr in range(PAIRS):
                        ps = pss[pr]
                        qs = st3.tile([128, 512], BF16, tag="qs")
                        nc.scalar.activation(qs[:], ps[:], AF.Identity)
                        rq = rp.tile([128, 512], F32, tag="rq")
                        nc.tensor.matmul(rq[:], ct["rmatT"][:], qs[:],
                                         start=True, stop=True)
                        rqs = st3.tile([128, 512], BF16, tag="rqs")
                        nc.scalar.activation(rqs[:], rq[:], AF.Identity)
                        t1 = st.tile([128, 512], BF16, tag="t1")
                        nc.vector.tensor_mul(t1[:], qs[:], ct["cos2"][:])
                        t2 = st.tile([128, 512], BF16, tag="t2")
                        nc.vector.tensor_mul(t2[:], rqs[:], ct["sin2"][:])
                        nc.vector.tensor_add(
                            dst[:, pr * 512:(pr + 1) * 512], t1[:], t2[:])
                    return dst

                for h in range(H):
                    qh = proj_rope("wq", h, "q")
                    kh = proj_rope("wk", h, "k")
                    for b in range(BPC):
                        # Causal skip: ktile1 (keys 128..255) is fully masked
                        # for queries 0..127, so score/exp/AV only cover
                        # [k0 x q-all | k1 x q-high] = 384 columns, not 512.
                        # em free layout: [0:256] = k-tile0 vs all 256 q;
                        # [256:384] = k-tile1 vs q in [128,256).
                        sc = scp.tile([128, 384], F32, tag="sc")
                        nc.tensor.matmul(
                            sc[:, 0:256],
                            kh[:, b * 256:b * 256 + 128],
                            qh[:, b * 256:(b + 1) * 256],
                            start=True, stop=True)
                        nc.tensor.matmul(
                            sc[:, 256:384],
                            kh[:, b * 256 + 128:b * 256 + 256],
                            qh[:, b * 256 + 128:b * 256 + 256],
                            start=True, stop=True)
                        ex = st3.tile([128, 384], BF16, tag="ex")
                        nc.scalar.activation(ex[:], sc[:], AF.Exp, scale=SCALE)
                        em = st3.tile([128, 384], BF16, tag="em")
                        tri = ct["mask2"][:, 0:128]       # k<=q triangle
                        nc.vector.tensor_mul(em[:, 0:128], ex[:, 0:128], tri)
                        nc.vector.tensor_copy(em[:, 128:256], ex[:, 128:256])
                        nc.vector.tensor_mul(em[:, 256:384], ex[:, 256:384], tri)
                        # softmax denominator on GpSimd (frees PE + a PSUM
                        # bank): all-reduce over partitions, combine the two
                        # k-tile column groups, reciprocal of row 0
                        red = st.tile([128, 384], F32, tag="red")
                        nc.gpsimd.partition_all_reduce(
                            red[:], em[:], channels=128,
                            reduce_op=bass_isa.ReduceOp.add)
                        srow = st.tile([1, 256], F32, tag="srow")
                        nc.vector.tensor_copy(srow[:, 0:128], red[0:1, 0:128])
                        nc.vector.tensor_add(srow[:, 128:256],
                                             red[0:1, 128:256],
                                             red[0:1, 256:384])
                        rr = st.tile([1, 256], F32R, tag="rr")
                        with nc.allow_low_precision(reason="f32r rhs for bcast mm"):
                            nc.vector.reciprocal(rr[:], srow[:])
                        # broadcast 1/sum across partitions on the idle GpSimd
                        # engine instead of a K=1 matmul + ScalarE copy
                        rbs = st.tile([128, 256], F32R, tag="rbs")
                        nc.gpsimd.partition_broadcast(rbs[:], rr[:])
                        yps = yp.tile([128, 256], F32, tag="yps")
                        nc.tensor.matmul(yps[:], vsb[b * 2][:, h * 128:(h + 1) * 128],
                                         em[:, 0:256], start=True, stop=False)
                        nc.tensor.matmul(yps[:, 128:256],
                                         vsb[b * 2 + 1][:, h * 128:(h + 1) * 128],
                                         em[:, 256:384], start=False, stop=True)
                        nc.vector.tensor_mul(ysb[h][:, b * 256:(b + 1) * 256],
                                             yps[:], rbs[:].bitcast(F32))

                # ---- output projection ----
                for o in range(16):
                    wb = ow.tile([128, KT * 128], BF16, tag="wob")
                    nc.sync.dma_start(wb[:], aps["wo"][o])
                    pss = [pp.tile([128, 512], F32, tag="pp", name=f"pso{pr}")
                           for pr in range(PAIRS)]
                    for c in range(KT):
                        for pr in range(PAIRS):
                            nc.tensor.matmul(
                                pss[pr][:], wb[:, c * 128:(c + 1) * 128],
                                ysb[c][:, pr * 512:(pr + 1) * 512],
                                start=(c == 0), stop=(c == KT - 1))
                    for pr in range(PAIRS):
                        os_ = st3.tile([128, 512], F32, tag="os")
                        nc.scalar.activation(os_[:], pss[pr][:], AF.Identity)
                        nc.sync.dma_start(
                            out_d[o * 128:(o + 1) * 128,
                                  pr * 512:(pr + 1) * 512], os_[:])

            if loop_n == 1:
                emit_body()
            else:
                hint = (mybir.EngineType.PE, mybir.EngineType.Activation,
                        mybir.EngineType.DVE, mybir.EngineType.SP,
                        mybir.EngineType.Pool)
                with tc.For_i(0, loop_n, 1, hint_engines=hint):
                    emit_body()
    nc.compile()
    return nc


def _host_consts():
    inv_freq = 1.0 / (10000.0 ** (np.arange(0, D, 2, dtype=np.float32) / D))
    t = np.arange(T, dtype=np.float32)
    freqs = np.outer(t, inv_freq)                      # [T, 64]
    emb = np.concatenate([freqs, freqs], axis=-1)      # [T, 128]
    cosT = np.cos(emb).T.astype(np.float32)            # [128, 256]
    sinT = np.sin(emb).T.astype(np.float32)
    cos2 = np.concatenate([cosT, cosT], axis=1)        # [128, 512]
    sin2 = np.concatenate([sinT, sinT], axis=1)
    rmat = np.zeros((128, 128), dtype=np.float32)      # R @ q = rotate_half(q)
    for d in range(64):
        rmat[d, d + 64] = -1.0
        rmat[d + 64, d] = 1.0
    rmatT = rmat.T.copy()
    mask2 = np.zeros((128, 512), dtype=np.float32)
    k_idx = np.arange(128)[:, None]
    q_idx = np.arange(256)[None, :]
    mask2[:, 0:256] = (k_idx <= q_idx).astype(np.float32)
    mask2[:, 256:512] = ((k_idx + 128) <= q_idx).astype(np.float32)
    return {
        "cos2": cos2.astype(NPBF), "sin2": sin2.astype(NPBF),
        "rmatT": rmatT.astype(NPBF), "mask2": mask2.astype(NPBF),
        "onescol": np.ones((128, 1), NPBF),
        "ones1x": np.ones((1, 128), np.float32),
    }


def _tile_w(w, blk):
    """w [2048,2048] fp32 -> [2048//blk, 128, 16*blk] bf16 with
    out[ob, p, kt*blk + n] = w[ob*blk + n, kt*128 + p]."""
    nblk = 2048 // blk
    t = w.reshape(nblk, blk, KT, 128).transpose(0, 3, 2, 1)
    return np.ascontiguousarray(t.reshape(nblk, 128, KT * blk)).astype(NPBF)


def _host_weights(wq, wk, wv, wo):
    return {
        "wq": _tile_w(np.asarray(wq, np.float32), 128),
        "wk": _tile_w(np.asarray(wk, np.float32), 128),
        "wv": _tile_w(np.asarray(wv, np.float32), 512),
        "wo": _tile_w(np.asarray(wo, np.float32), 128),
    }


def _host_x(x):
    """x [B,T,C] fp32 -> per-core [KT, 128, BPC*T] bf16 list."""
    xb = np.asarray(x, np.float32).astype(NPBF)
    outs = []
    for c in range(NCORES):
        xs = xb[c * BPC:(c + 1) * BPC]                      # [4,256,2048]
        xt = xs.transpose(2, 0, 1).reshape(KT, 128, BPC * T)
        outs.append(np.ascontiguousarray(xt))
    return outs


def kernel(x, wq, wk, wv, wo):
    if "nc" not in _CACHE:
        _CACHE["nc"] = _build()
    nc = _CACHE["nc"]
    shared = {**_host_weights(wq, wk, wv, wo), **_host_consts()}
    xts = _host_x(x)
    in_maps = [{"xt": xts[c], **shared} for c in range(NCORES)]
    res = run_bass_kernel_spmd(nc, in_maps, core_ids=list(range(NCORES)))
    outs = []
    for c in range(NCORES):
        o = res.results[c]["out"]                      # [2048, 1024]
        o = o.reshape(C, PAIRS, 2, T)                  # [c, pair, b, t]
        o = o.transpose(1, 2, 3, 0).reshape(BPC, T, C)
        outs.append(o)
    return np.concatenate(outs, axis=0).astype(np.float32)


# revision 18
# speedup vs baseline: 1.0601x; 1.0601x over previous
"""LLaMA attention block on 8 Trainium2 NeuronCores (Bass/Tile).

Problem: x[32,256,2048], wq/wk/wv/wo[2048,2048] fp32.
  q/k/v = x@W.T (per-head RoPE on q,k), causal softmax attention, y@wo.T.

Strategy (v3):
- Data-parallel over batch: 8 cores x 4 batch elements (1024 tokens/core).
- bf16 operands everywhere on the matmul path (weights, x, q/k/v, exp-scores,
  y), fp32 PSUM accumulation. Measured end-to-end error vs fp64 ~6e-3,
  well under the 2e-2 gate. PE streams 1 col/cycle for bf16 == f32r, so this
  costs no PE time but halves DMA and SBUF traffic.
- No DRAM spill: q/k (rotated) live in small rotating SBUF pools; v and y
  (bf16, 4MB each) persist in SBUF. Per-head loop interleaves
  {q-proj, k-proj, attention} so attention overlaps the next head's
  projection matmuls and the PE never drains.
- All pools are allocated up-front at disjoint SBUF addresses: scoped/reused
  address ranges were observed (TimelineSim) to serialize weight prefetch
  behind the previous phase (a 4-8us PE gap per phase boundary).
- x is loaded as 16 separate tiles so the first V matmuls start after ~2us.
- Weights are pre-tiled on the host into the exact SBUF layout so every
  weight DMA is fully contiguous.
- "Transposed" activation layout on-chip ([contraction, tokens]); softmax
  reduction via ones-matmul; normalization via K=1 broadcast matmul.
- RoPE: rotate-half as one 128x128 matmul per tile; elementwise
  (q*cos + rq*sin) in bf16 on DVE (4x mode); PSUM drains on ScalarE.
- _build(loop_n=N) wraps the whole body in a tc.For_i hardware loop: one
  NEFF execution runs the kernel N times back-to-back. Used by test.py to
  measure pure on-device time as a slope between two loop counts,
  cancelling host/axon dispatch latency.
"""
import sys
sys.path.insert(0, '/opt/trn_rl_repo')
import math
import numpy as np
import ml_dtypes

import concourse.bass as bass
import concourse.bass_isa as bass_isa
import concourse.bacc as bacc
import concourse.mybir as mybir
import concourse.tile as tile
from concourse.bass_utils import run_bass_kernel_spmd

B, T, C = 32, 256, 2048
H, D = 16, 128
NCORES = 8
BPC = B // NCORES           # 4 batches per core
PAIRS = BPC // 2            # 2 batch-pairs (N=512 matmuls)
KT = C // 128               # 16 contraction tiles
SCALE = 1.0 / math.sqrt(D)

F32 = mybir.dt.float32
F32R = mybir.dt.float32r
BF16 = mybir.dt.bfloat16
AF = mybir.ActivationFunctionType
NPBF = ml_dtypes.bfloat16

_CACHE = {}


def _build(loop_n: int = 1):
    nc = bacc.Bacc("TRN2", target_bir_lowering=False, debug=False, num_devices=1)
    dt_in = {
        "xt": ([KT, 128, BPC * T], BF16),      # [kt, p, (b t)]
        "wq": ([16, 128, KT * 128], BF16),     # [o_blk, p, (kt n)]
        "wk": ([16, 128, KT * 128], BF16),
        "wv": ([4, 128, KT * 512], BF16),      # [og, p, (kt n)]
        "wo": ([16, 128, KT * 128], BF16),
        "rmatT": ([128, 128], BF16),
        "cos2": ([128, 512], BF16),
        "sin2": ([128, 512], BF16),
        "mask2": ([128, 512], BF16),
        "onescol": ([128, 1], BF16),
        "ones1x": ([1, 128], F32R),
    }
    aps = {n: nc.dram_tensor(n, s, d, kind="ExternalInput").ap()
           for n, (s, d) in dt_in.items()}
    out_d = nc.dram_tensor("out", [C, BPC * T], F32, kind="ExternalOutput").ap()

    with tile.TileContext(nc) as tc:
        from contextlib import ExitStack
        with ExitStack() as top:
            # ---- constants (loaded once, outside any timing loop) ----
            cpool = top.enter_context(tc.tile_pool(name="const", bufs=1))
            ct = {}
            for n in ("rmatT", "cos2", "sin2", "mask2", "onescol", "ones1x"):
                shape = dt_in[n][0]
                ct[n] = cpool.tile(list(shape), dt_in[n][1], tag=n, name=n)
                nc.sync.dma_start(ct[n][:], aps[n][:])

            # ---- all pools up-front: disjoint SBUF regions ----
            xpool = top.enter_context(tc.tile_pool(name="xt", bufs=1))
            xtiles = [xpool.tile([128, BPC * T], BF16, tag=f"xt{kt}",
                                 name=f"xt{kt}") for kt in range(KT)]
            vpool = top.enter_context(tc.tile_pool(name="v", bufs=1))
            vsb = [vpool.tile([128, C], BF16, tag=f"v{i}", name=f"v{i}")
                   for i in range(BPC * 2)]       # [tok(128), all head dims]
            ypool = top.enter_context(tc.tile_pool(name="y", bufs=1))
            ysb = [ypool.tile([128, BPC * T], BF16, tag=f"y{h}", name=f"y{h}")
                   for h in range(H)]             # [head dims(128), tok]
            qkpool = top.enter_context(tc.tile_pool(name="qk", bufs=4))
            st = top.enter_context(tc.tile_pool(name="st", bufs=2))
            st3 = top.enter_context(tc.tile_pool(name="st3", bufs=3))
            vw = top.enter_context(tc.tile_pool(name="vw", bufs=4))
            qkw = top.enter_context(tc.tile_pool(name="qkw", bufs=3))
            ow = top.enter_context(tc.tile_pool(name="ow", bufs=2))

            # ---- PSUM pools: exactly 8 banks ----
            pp = top.enter_context(tc.tile_pool(name="pp", bufs=3, space="PSUM"))
            rp = top.enter_context(tc.tile_pool(name="rp", bufs=1, space="PSUM"))
            scp = top.enter_context(tc.tile_pool(name="scp", bufs=2, space="PSUM"))
            yp = top.enter_context(tc.tile_pool(name="yp", bufs=1, space="PSUM"))
            sep = top.enter_context(tc.tile_pool(name="sep", bufs=1, space="PSUM"))

            def emit_body():
                # First V matmul needs wv[0] first-half + xt[0]; queue those
                # bytes ahead of the rest of x so the PE starts ~3.5us in.
                wv_first = vw.tile([128, 8 * 512], BF16, tag="wvb")
                nc.sync.dma_start(wv_first[:], aps["wv"][0, :, 0:4096])
                for kt in range(KT):
                    nc.sync.dma_start(xtiles[kt][:], aps["xt"][kt])

                # ---- V projection: v in [tok, C] layout, 4-head groups,
                #      each og's weight split in two tiles for finer DMA deps
                for og in range(4):
                    wbs = []
                    for half in range(2):
                        if og == 0 and half == 0:
                            wbs.append(wv_first)
                            continue
                        wb = vw.tile([128, 8 * 512], BF16, tag="wvb")
                        nc.sync.dma_start(
                            wb[:], aps["wv"][og, :, half * 4096:(half + 1) * 4096])
                        wbs.append(wb)
                    for tt in range(BPC * 2):
                        ps = pp.tile([128, 512], F32, tag="pp")
                        for kt in range(KT):
                            nc.tensor.matmul(
                                ps[:],
                                xtiles[kt][:, tt * 128:(tt + 1) * 128],
                                wbs[kt // 8][:, (kt % 8) * 512:(kt % 8 + 1) * 512],
                                start=(kt == 0), stop=(kt == KT - 1))
                        nc.scalar.activation(
                            vsb[tt][:, og * 512:(og + 1) * 512], ps[:], AF.Identity)

                # ---- per-head: q-proj+rope, k-proj+rope, attention ----
                def proj_rope(wname, h, tag):
                    wb = qkw.tile([128, KT * 128], BF16, tag="wqk")
                    nc.sync.dma_start(wb[:], aps[wname][h])
                    dst = qkpool.tile([128, BPC * T], BF16, tag=tag)
                    # kt-outer so each 128-col weight slice is loaded into the
                    # PE once and streamed against both batch-pairs.
                    pss = [pp.tile([128, 512], F32, tag="pp", name=f"ps{pr}")
                           for pr in range(PAIRS)]
                    for kt in range(KT):
                        for pr in range(PAIRS):
                            nc.tensor.matmul(
                                pss[pr][:], wb[:, kt * 128:(kt + 1) * 128],
                                xtiles[kt][:, pr * 512:(pr + 1) * 512],
                                start=(kt == 0), stop=(kt == KT - 1))
                    for pr in range(PAIRS):
                        ps = pss[pr]
                        qs = st3.tile([128, 512], BF16, tag="qs")
                        nc.scalar.activation(qs[:], ps[:], AF.Identity)
                        rq = rp.tile([128, 512], F32, tag="rq")
                        nc.tensor.matmul(rq[:], ct["rmatT"][:], qs[:],
                                         start=True, stop=True)
                        rqs = st3.tile([128, 512], BF16, tag="rqs")
                        nc.scalar.activation(rqs[:], rq[:], AF.Identity)
                        t1 = st.tile([128, 512], BF16, tag="t1")
                        nc.vector.tensor_mul(t1[:], qs[:], ct["cos2"][:])
                        t2 = st.tile([128, 512], BF16, tag="t2")
                        nc.vector.tensor_mul(t2[:], rqs[:], ct["sin2"][:])
                        nc.vector.tensor_add(
                            dst[:, pr * 512:(pr + 1) * 512], t1[:], t2[:])
                    return dst

                for h in range(H):
                    qh = proj_rope("wq", h, "q")
                    kh = proj_rope("wk", h, "k")
                    for b in range(BPC):
                        # Causal skip: ktile1 (keys 128..255) is fully masked
                        # for queries 0..127, so score/exp/AV only cover
                        # [k0 x q-all | k1 x q-high] = 384 columns, not 512.
                        # em free layout: [0:256] = k-tile0 vs all 256 q;
                        # [256:384] = k-tile1 vs q in [128,256).
                        sc = scp.tile([128, 384], F32, tag="sc")
                        nc.tensor.matmul(
                            sc[:, 0:256],
                            kh[:, b * 256:b * 256 + 128],
                            qh[:, b * 256:(b + 1) * 256],
                            start=True, stop=True)
                        nc.tensor.matmul(
                            sc[:, 256:384],
                            kh[:, b * 256 + 128:b * 256 + 256],
                            qh[:, b * 256 + 128:b * 256 + 256],
                            start=True, stop=True)
                        ex = st3.tile([128, 384], BF16, tag="ex")
                        nc.scalar.activation(ex[:], sc[:], AF.Exp, scale=SCALE)
                        em = st3.tile([128, 384], BF16, tag="em")
                        tri = ct["mask2"][:, 0:128]       # k<=q triangle
                        nc.vector.tensor_mul(em[:, 0:128], ex[:, 0:128], tri)
                        nc.vector.tensor_copy(em[:, 128:256], ex[:, 128:256])
                        nc.vector.tensor_mul(em[:, 256:384], ex[:, 256:384], tri)
                        se = sep.tile([1, 256], F32, tag="serb")
                        nc.tensor.matmul(se[:], ct["onescol"][:], em[:, 0:256],
                                         start=True, stop=False)
                        nc.tensor.matmul(se[:, 128:256], ct["onescol"][:],
                                         em[:, 256:384],
                                         start=False, stop=True)
                        rr = st.tile([1, 256], F32R, tag="rr")
                        with nc.allow_low_precision(reason="f32r rhs for bcast mm"):
                            nc.vector.reciprocal(rr[:], se[:])
                        # broadcast 1/sum across partitions on the idle GpSimd
                        # engine instead of a K=1 matmul + ScalarE copy
                        rbs = st.tile([128, 256], F32R, tag="rbs")
                        nc.gpsimd.partition_broadcast(rbs[:], rr[:])
                        yps = yp.tile([128, 256], F32, tag="yps")
                        nc.tensor.matmul(yps[:], vsb[b * 2][:, h * 128:(h + 1) * 128],
                                         em[:, 0:256], start=True, stop=False)
                        nc.tensor.matmul(yps[:, 128:256],
                                         vsb[b * 2 + 1][:, h * 128:(h + 1) * 128],
                                         em[:, 256:384], start=False, stop=True)
                        nc.vector.tensor_mul(ysb[h][:, b * 256:(b + 1) * 256],
                                             yps[:], rbs[:].bitcast(F32))

                # ---- output projection ----
                for o in range(16):
                    wb = ow.tile([128, KT * 128], BF16, tag="wob")
                    nc.sync.dma_start(wb[:], aps["wo"][o])
                    pss = [pp.tile([128, 512], F32, tag="pp", name=f"pso{pr}")
                           for pr in range(PAIRS)]
                    for c in range(KT):
                        for pr in range(PAIRS):
                            nc.tensor.matmul(
                                pss[pr][:], wb[:, c * 128:(c + 1) * 128],
                                ysb[c][:, pr * 512:(pr + 1) * 512],
                                start=(c == 0), stop=(c == KT - 1))
                    for pr in range(PAIRS):
                        os_ = st3.tile([128, 512], F32, tag="os")
                        nc.scalar.activation(os_[:], pss[pr][:], AF.Identity)
                        nc.sync.dma_start(
                            out_d[o * 128:(o + 1) * 128,
                                  pr * 512:(pr + 1) * 512], os_[:])

            if loop_n == 1:
                emit_body()
            else:
                hint = (mybir.EngineType.PE, mybir.EngineType.Activation,
                        mybir.EngineType.DVE, mybir.EngineType.SP,
                        mybir.EngineType.Pool)
                with tc.For_i(0, loop_n, 1, hint_engines=hint):
                    emit_body()
    nc.compile()
    return nc


def _host_consts():
    inv_freq = 1.0 / (10000.0 ** (np.arange(0, D, 2, dtype=np.float32) / D))
    t = np.arange(T, dtype=np.float32)
    freqs = np.outer(t, inv_freq)                      # [T, 64]
    emb = np.concatenate([freqs, freqs], axis=-1)      # [T, 128]
    cosT = np.cos(emb).T.astype(np.float32)            # [128, 256]
    sinT = np.sin(emb).T.astype(np.float32)
    cos2 = np.concatenate([cosT, cosT], axis=1)        # [128, 512]
    sin2 = np.concatenate([sinT, sinT], axis=1)
    rmat = np.zeros((128, 128), dtype=np.float32)      # R @ q = rotate_half(q)
    for d in range(64):
        rmat[d, d + 64] = -1.0
        rmat[d + 64, d] = 1.0
    rmatT = rmat.T.copy()
    mask2 = np.zeros((128, 512), dtype=np.float32)
    k_idx = np.arange(128)[:, None]
    q_idx = np.arange(256)[None, :]
    mask2[:, 0:256] = (k_idx <= q_idx).astype(np.float32)
    mask2[:, 256:512] = ((k_idx + 128) <= q_idx).astype(np.float32)
    return {
        "cos2": cos2.astype(NPBF), "sin2": sin2.astype(NPBF),
        "rmatT": rmatT.astype(NPBF), "mask2": mask2.astype(NPBF),
        "onescol": np.ones((128, 1), NPBF),
        "ones1x": np.ones((1, 128), np.float32),
    }


def _tile_w(w, blk):
    """w [2048,2048] fp32 -> [2048//blk, 128, 16*blk] bf16 with
    out[ob, p, kt*blk + n] = w[ob*blk + n, kt*128 + p]."""
    nblk = 2048 // blk
    t = w.reshape(nblk, blk, KT, 128).transpose(0, 3, 2, 1)
    return np.ascontiguousarray(t.reshape(nblk, 128, KT * blk)).astype(NPBF)


def _host_weights(wq, wk, wv, wo):
    return {
        "wq": _tile_w(np.asarray(wq, np.float32), 128),
        "wk": _tile_w(np.asarray(wk, np.float32), 128),
        "wv": _tile_w(np.asarray(wv, np.float32), 512),
        "wo": _tile_w(np.asarray(wo, np.float32), 128),
    }


def _host_x(x):
    """x [B,T,C] fp32 -> per-core [KT, 128, BPC*T] bf16 list."""
    xb = np.asarray(x, np.float32).astype(NPBF)
    outs = []
    for c in range(NCORES):
        xs = xb[c * BPC:(c + 1) * BPC]                      # [4,256,2048]
        xt = xs.transpose(2, 0, 1).reshape(KT, 128, BPC * T)
        outs.append(np.ascontiguousarray(xt))
    return outs


def kernel(x, wq, wk, wv, wo):
    if "nc" not in _CACHE:
        _CACHE["nc"] = _build()
    nc = _CACHE["nc"]
    shared = {**_host_weights(wq, wk, wv, wo), **_host_consts()}
    xts = _host_x(x)
    in_maps = [{"xt": xts[c], **shared} for c in range(NCORES)]
    res = run_bass_kernel_spmd(nc, in_maps, core_ids=list(range(NCORES)))
    outs = []
    for c in range(NCORES):
        o = res.results[c]["out"]                      # [2048, 1024]
        o = o.reshape(C, PAIRS, 2, T)                  # [c, pair, b, t]
        o = o.transpose(1, 2, 3, 0).reshape(BPC, T, C)
        outs.append(o)
    return np.concatenate(outs, axis=0).astype(np.float32)
